# revision 8
# baseline (speedup 1.0000x reference)
"""BBoxEncoder Trainium2 kernel (v3).

Per ray r, BVH level l (8 levels), the reference gathers an embedding row
f = bbox_emb[history[r, l]] (8 corners x 32 dims), normalizes the ray's 16
sample points into the node's AABB, builds trilinear corner weights
w[p, c] and emits feat[r, l, p, d] = sum_c w[p, c] * f[c, d].

Strategy (data-parallel over rays, 8 NeuronCores):
  - shard inp/history along axis 0, replicate the embedding table (fp16)
  - host precomputes per-(ray, level) [nmin | 1/extent] (fp32, small
    gather from the 1.5 MB AABB tables) so the device streams it
  - J=4 rays per SBUF partition: per 512-ray tile, 32 indirect-DMA
    gathers (one offset column each -- the HW DGE consumes exactly one
    offset per partition per instruction), coordinate math in fp32,
    weights + 8-corner MAC in fp16, contiguous fp16 store
  - host upcasts the fp16 output to fp32

kernel(**inputs) takes the FULL unsharded inputs and returns the FULL
(32768, 4096) float32 output.
"""

import os as _os

import numpy as np
from contextlib import ExitStack

import concourse.bass as bass
import concourse.tile as tile
from concourse import bacc, mybir
from concourse.bass import IndirectOffsetOnAxis, ts
from concourse.bass_utils import run_bass_kernel_spmd

F32 = mybir.dt.float32
F16 = mybir.dt.float16
I32 = mybir.dt.int32
AL = mybir.AluOpType

ENC_DEPTH = 8
N_POINTS = 16
ENC_DIM = 32

N_CORES = 8
N_RAYS = 32768
N_NODES = 65536

J = int(_os.environ.get("KERNEL_RPP", "4"))  # rays per partition

# Tensors replicated across cores (read by sibling tools).
REPLICATED = {"emb"}

# corner order used on-device: c = bx*4 + by*2 + bz (x-bit major).
# reference order (torch chunk order): 000,100,010,001,101,011,110,111
_REF_CORNERS = [
    (0, 0, 0), (1, 0, 0), (0, 1, 0), (0, 0, 1),
    (1, 0, 1), (0, 1, 1), (1, 1, 0), (1, 1, 1),
]
PERM = [0] * 8
for _i, (_bx, _by, _bz) in enumerate(_REF_CORNERS):
    PERM[_bx * 4 + _by * 2 + _bz] = _i


def _emit(ctx: ExitStack, tc, io, n_shard, n_levels):
    """Emit the per-core program. io: dict of DRAM tensor handles."""
    nc = tc.nc
    P = 128
    L = n_levels
    JL = J * L
    rays_per_tile = P * J
    n_tiles = n_shard // rays_per_tile
    PD = N_POINTS * ENC_DIM  # 512
    OUT_W = ENC_DEPTH * PD  # 4096

    inp_d = io["inp"].ap()        # (n_shard, 48) f32
    hist_d = io["hist"].ap()      # (n_shard, 8) int32
    geo_d = io["geo"].ap()        # (n_shard, L*6) f32  [nmin | inv_ext]
    emb_d = io["emb"]             # (N_NODES, 256) f16, corner-permuted
    out_d = io["out"].ap()        # (n_shard, 4096) f16

    ld = ctx.enter_context(tc.tile_pool(name="ld", bufs=3))
    gat = ctx.enter_context(tc.tile_pool(name="gat", bufs=3))
    wrk = ctx.enter_context(tc.tile_pool(name="wrk", bufs=2))
    tmp_p = ctx.enter_context(tc.tile_pool(name="tmpp", bufs=1))
    acc_p = ctx.enter_context(tc.tile_pool(name="acc", bufs=2))

    for i in range(n_tiles):
        r0 = i * rays_per_tile
        inp_t = ld.tile([P, J * 48], F32, tag="inp")
        nc.sync.dma_start(inp_t[:], inp_d[r0:r0 + rays_per_tile, :])
        hist_t = ld.tile([P, J * ENC_DEPTH], I32, tag="hist")
        nc.sync.dma_start(hist_t[:], hist_d[r0:r0 + rays_per_tile, :])
        geo_t = ld.tile([P, J * L * 6], F32, tag="geo")
        nc.sync.dma_start(geo_t[:], geo_d[r0:r0 + rays_per_tile, :])

        # gathers: one offset column per (j, l); chunk jl = j*L + l
        f_t = gat.tile([P, JL * 256], F16, tag="f")
        for j in range(J):
            for l in range(L):
                col = j * ENC_DEPTH + l
                jl = j * L + l
                nc.gpsimd.indirect_dma_start(
                    out=f_t[:, jl * 256:(jl + 1) * 256],
                    out_offset=None,
                    in_=emb_d.ap(),
                    in_offset=IndirectOffsetOnAxis(
                        ap=hist_t[:, col:col + 1], axis=0),
                )

        g_v = geo_t[:].rearrange("q (j l e) -> q j l e", j=J, e=6)
        g_jl = geo_t[:].rearrange("q (jl e) -> q jl e", e=6)  # (P, JL, 6)

        # x[q, jl, p, e] = clip((inp[q, j, p, e] - nmin[q, jl, e]) * inv)
        x_t = wrk.tile([P, JL * 48], F32, tag="x")
        x_v = x_t[:].rearrange(
            "q (j l p e) -> q j l p e", j=J, p=N_POINTS, e=3)
        x_jl = x_t[:].rearrange(
            "q (jl p e) -> q jl p e", p=N_POINTS, e=3)
        for j in range(J):
            inp_b = (
                inp_t[:, j * 48:(j + 1) * 48]
                .rearrange("q (p e) -> q p e", e=3)
                .unsqueeze(1).to_broadcast([P, L, N_POINTS, 3])
            )
            nmin_b = (g_v[:, j, :, 0:3].unsqueeze(2)
                      .to_broadcast([P, L, N_POINTS, 3]))
            nc.vector.tensor_tensor(
                out=x_v[:, j], in0=inp_b, in1=nmin_b, op=AL.subtract)
        inv_b = (g_jl[:, :, 3:6].unsqueeze(2)
                 .to_broadcast([P, JL, N_POINTS, 3]))
        nc.vector.tensor_tensor(out=x_jl, in0=x_jl, in1=inv_b, op=AL.mult)
        nc.vector.tensor_scalar(
            out=x_t[:], in0=x_t[:], scalar1=0.0, scalar2=1.0,
            op0=AL.max, op1=AL.min,
        )

        # ft[q, axis, jl, s, p]: s=0 -> 1-t, s=1 -> t   (fp16)
        ft_t = wrk.tile([P, 3 * JL * 2 * N_POINTS], F16, tag="ft")
        ft_v = ft_t[:].rearrange(
            "q (a jl s p) -> q a jl s p", a=3, s=2, p=N_POINTS
        )
        x_w = x_jl.transpose([0, 3, 1, 2])          # (P, 3, JL, 16) view
        nc.vector.tensor_scalar(
            out=ft_v[:, :, :, 0, :].transpose([0, 2, 3, 1]),
            in0=x_w.transpose([0, 2, 3, 1]),
            scalar1=-1.0, scalar2=1.0, op0=AL.mult, op1=AL.add,
        )
        nc.vector.tensor_copy(
            out=ft_v[:, :, :, 1, :].transpose([0, 2, 3, 1]),
            in_=x_w.transpose([0, 2, 3, 1]),
        )

        # wxy[q, jl, bx, by, p] then w[q, jl, bx, by, bz, p]   (fp16)
        wxy_t = wrk.tile([P, JL * 4 * N_POINTS], F16, tag="wxy")
        wxy_v = wxy_t[:].rearrange(
            "q (jl x y p) -> q jl x y p", x=2, y=2, p=N_POINTS
        )
        for bx in range(2):
            nc.vector.tensor_tensor(
                out=wxy_v[:, :, bx],
                in0=ft_v[:, 0, :, bx, :].unsqueeze(2)
                    .to_broadcast([P, JL, 2, N_POINTS]),
                in1=ft_v[:, 1],
                op=AL.mult,
            )
        w_t = wrk.tile([P, JL * 8 * N_POINTS], F16, tag="w")
        w_v = w_t[:].rearrange(
            "q (jl x y z p) -> q jl x y z p", x=2, y=2, z=2, p=N_POINTS
        )
        for bx in range(2):
            for by in range(2):
                nc.vector.tensor_tensor(
                    out=w_v[:, :, bx, by],
                    in0=wxy_v[:, :, bx, by, :].unsqueeze(2)
                        .to_broadcast([P, JL, 2, N_POINTS]),
                    in1=ft_v[:, 2],
                    op=AL.mult,
                )
        w_c = w_t[:].rearrange("q (jl c p) -> q jl c p", c=8, p=N_POINTS)
        f_c = f_t[:].rearrange("q (jl c d) -> q jl c d", c=8, d=ENC_DIM)

        # acc[q, jl, p, d] = sum_c w[q, jl, c, p] * f[q, jl, c, d]   (fp16)
        acc_t = acc_p.tile([P, J * OUT_W], F16, tag="acc")
        acc_v = acc_t[:].rearrange(
            "q (j lw) -> q j lw", j=J)[:, :, : L * PD].rearrange(
            "q j (l p d) -> q (j l) p d", p=N_POINTS, d=ENC_DIM) \
            if L < ENC_DEPTH else \
            acc_t[:].rearrange(
                "q (jl p d) -> q jl p d", p=N_POINTS, d=ENC_DIM)
        tmp_t = tmp_p.tile([P, JL * PD], F16, tag="tmp")
        tmp_v = tmp_t[:].rearrange(
            "q (jl p d) -> q jl p d", p=N_POINTS, d=ENC_DIM
        )
        for c in range(8):
            dst = acc_v if c == 0 else tmp_v
            nc.vector.tensor_tensor(
                out=dst,
                in0=w_c[:, :, c, :].unsqueeze(3)
                    .to_broadcast([P, JL, N_POINTS, ENC_DIM]),
                in1=f_c[:, :, c, :].unsqueeze(2)
                    .to_broadcast([P, JL, N_POINTS, ENC_DIM]),
                op=AL.mult,
            )
            if c > 0:
                nc.vector.tensor_tensor(
                    out=acc_v, in0=acc_v, in1=tmp_v, op=AL.add
                )
        if L < ENC_DEPTH:
            av = acc_t[:].rearrange("q (j lw) -> q j lw", j=J)
            nc.gpsimd.memset(av[:, :, L * PD:], 0.0)

        nc.sync.dma_start(out_d[r0:r0 + rays_per_tile, :], acc_t[:])


def build_program(n_shard, n_nodes, n_levels):
    nc = bacc.Bacc(
        "TRN2", target_bir_lowering=False, debug=False, enable_asserts=False
    )
    io = {
        "inp": nc.dram_tensor("inp", [n_shard, 48], F32, kind="ExternalInput"),
        "hist": nc.dram_tensor("hist", [n_shard, ENC_DEPTH], I32,
                               kind="ExternalInput"),
        "geo": nc.dram_tensor("geo", [n_shard, n_levels * 6], F32,
                              kind="ExternalInput"),
        "emb": nc.dram_tensor("emb", [n_nodes, 256], F16,
                              kind="ExternalInput"),
        "out": nc.dram_tensor(
            "out", [n_shard, ENC_DEPTH * N_POINTS * ENC_DIM], F16,
            kind="ExternalOutput",
        ),
    }
    with tile.TileContext(nc) as tc, ExitStack() as ctx:
        _emit(ctx, tc, io, n_shard, n_levels)
    nc.compile()
    return nc


_CACHE = {}


def _get_program(n_shard, n_nodes, n_levels):
    key = (n_shard, n_nodes, n_levels)
    if key not in _CACHE:
        _CACHE[key] = build_program(n_shard, n_nodes, n_levels)
    return _CACHE[key]


def make_in_maps(inp, history, bbox_emb, nodes_min, nodes_max, n_cores=N_CORES,
                 n_levels=ENC_DEPTH):
    """Host-side marshalling: shard rays, permute emb corners, gather geo."""
    n_rays = inp.shape[0]
    shard = n_rays // n_cores
    L = n_levels
    inp_f = np.ascontiguousarray(
        inp.reshape(n_rays, 48).astype(np.float32, copy=False)
    )
    hist8 = np.ascontiguousarray(
        history[:, :ENC_DEPTH].astype(np.int32, copy=False))
    nmin = nodes_min.astype(np.float32, copy=False)
    ext = (nodes_max - nodes_min).astype(np.float32, copy=False)
    ext = np.where(ext == 0, np.float32(0.5), ext)
    geo_tab = np.concatenate([nmin, (1.0 / ext).astype(np.float32)], axis=1)
    geo = np.ascontiguousarray(
        geo_tab[hist8[:, :L]].reshape(n_rays, L * 6))
    n_nodes = nodes_min.shape[0]
    emb_p = np.ascontiguousarray(
        bbox_emb.astype(np.float32, copy=False)
        .reshape(n_nodes, 8, ENC_DIM)[:, PERM, :]
        .reshape(n_nodes, 8 * ENC_DIM)
        .astype(np.float16)
    )
    in_maps = []
    for c in range(n_cores):
        sl = slice(c * shard, (c + 1) * shard)
        in_maps.append({
            "inp": inp_f[sl],
            "hist": hist8[sl],
            "geo": geo[sl],
            "emb": emb_p,
        })
    return in_maps, shard, n_nodes


def kernel(inp, history, depth, bbox_emb, nodes_min, nodes_max):
    inp = np.asarray(inp)
    history = np.asarray(history)
    depth = np.asarray(depth)
    bbox_emb = np.asarray(bbox_emb)
    nodes_min = np.asarray(nodes_min)
    nodes_max = np.asarray(nodes_max)

    n_rays = inp.shape[0]
    n_levels = int(min(int(depth.max()), ENC_DEPTH)) if depth.size else 0
    if n_levels <= 0:
        return np.zeros((n_rays, ENC_DEPTH * N_POINTS * ENC_DIM), np.float32)
    in_maps, shard, n_nodes = make_in_maps(
        inp, history, bbox_emb, nodes_min, nodes_max, n_levels=n_levels
    )

    nc = _get_program(shard, n_nodes, n_levels)
    res = run_bass_kernel_spmd(nc, in_maps, core_ids=list(range(N_CORES)))
    out = np.concatenate(
        [r["out"].astype(np.float32) for r in res.results], axis=0)
    return out
